# revision 1
# baseline (speedup 1.0000x reference)
"""Multi-head attention (B=4, S=2048, D=1024, H=16) on 8 trn2 NeuronCores.

Sharding: batch x head-half. Core c handles batch b = c//2 and heads
hh*8..hh*8+8 where hh = c%2. Each core computes its heads' Q/K/V
projections, attention, and a partial output projection; the host sums
the two partials per batch and adds the (constant) bias terms.

Device-side layout choices (per core, S=2048, DH=512 head dims):
  q_dT, k_dT : [depth-dims, S]  (f32r, matmul-ready: contraction on partitions)
  scores_T   : [Sk, Sq] tiles  = k_dT.T @ q_dT  (K=64, head pairs packed into
               PE row-groups 0-1 / 2-3; pipelined one skc ahead of exp)
  attn       : exp on ACT (PSUM->SBUF, bf16), multiplicative binary mask on DVE
  PV swap    : o[q, d] += P_chunk.T @ v_aug with P (the attn weights) as the
               128-wide stationary operand — halves PE rows vs stationary-V.
               v_aug col 64 is all-ones so o[:, 64] is the softmax denominator.
  normalize  : per-partition reciprocal + scalar-broadcast multiply on DVE
               (q sits on partitions, so no cross-partition broadcast needed)
  transpose  : o_n[q, d] -> oT[d, q] via PE transpose-mode (identity operand),
               deferred into the next head-pair's loop as PE filler
  out proj   : out[Sq, 1024] = o_T.T @ WoT  (bf16), DMA to DRAM
  x/w DMA    : inputs and projection weights staged as bf16 (halves HBM
               traffic); K(st>0) / Q(st0, mc>0) projections are spliced
               just-in-time into st0's four head-pair loops so the exp
               stream starts ~26us in instead of ~66us

Scale 1/sqrt(depth) is folded into Wq/bq on the host. bv and bo are folded
into a constant host-side bias (attention rows sum to 1).
"""

import numpy as np

D = 1024
S = 2048
HPC = 8          # heads per core
DH = HPC * 64    # 512 per-core head dims
N_CORES = 8

_CACHE = {}


def _build_program(reps=1):
    if reps in _CACHE:
        return _CACHE[reps]

    from concourse import bacc, tile, mybir

    f32 = mybir.dt.float32
    f32r = mybir.dt.float32r
    bf16 = mybir.dt.bfloat16
    f8 = mybir.dt.float8e4
    AF = mybir.ActivationFunctionType

    nc = bacc.Bacc(
        "TRN2",
        target_bir_lowering=False,
        debug=False,
        enable_asserts=False,
        num_devices=N_CORES,
    )

    xqT = nc.dram_tensor("xqT", [D, S], bf16, kind="ExternalInput").ap()
    xkT = nc.dram_tensor("xkT", [D, S], bf16, kind="ExternalInput").ap()
    xvT = nc.dram_tensor("xvT", [D, S], bf16, kind="ExternalInput").ap()
    ident_d = nc.dram_tensor("ident", [128, 128], bf16, kind="ExternalInput").ap()
    wqT = nc.dram_tensor("wqT", [D, DH], bf16, kind="ExternalInput").ap()
    wkT = nc.dram_tensor("wkT", [D, DH], bf16, kind="ExternalInput").ap()
    wvT = nc.dram_tensor("wvT", [D, DH], bf16, kind="ExternalInput").ap()
    woT = nc.dram_tensor("woT", [DH, D], bf16, kind="ExternalInput").ap()
    bq_d = nc.dram_tensor("bq", [DH], f32, kind="ExternalInput").ap()
    bk_d = nc.dram_tensor("bk", [DH], f32, kind="ExternalInput").ap()
    mmul = nc.dram_tensor("maskmul", [S, S], f8, kind="ExternalInput").ap()
    out = nc.dram_tensor("out", [S, D], bf16, kind="ExternalOutput").ap()

    with tile.TileContext(nc) as tc:
        with (
            nc.allow_low_precision(reason="f32r is fp32-width; rounding intended"),
            tc.tile_pool(name="big", bufs=1) as big,
            tc.tile_pool(name="ot", bufs=2) as otp,
            tc.tile_pool(name="wp", bufs=3) as wp,
            tc.tile_pool(name="stream", bufs=2) as stream,
            tc.tile_pool(name="xtp", bufs=2) as xtp,
            tc.tile_pool(name="mp", bufs=2) as mp,
            tc.tile_pool(name="small", bufs=3) as small,
            tc.tile_pool(name="aux", bufs=4) as auxp,
            tc.tile_pool(name="ps", bufs=2, space="PSUM") as psp,
            tc.tile_pool(name="po", bufs=2, space="PSUM") as pop,
        ):
            # ---- persistent tiles ----
            q_dT = big.tile([128, 4, S], bf16, tag="q_dT")
            k_dT = big.tile([128, 4, S], bf16, tag="k_dT")
            v_sb = big.tile([128, 16, HPC, 65], bf16, tag="v_sb")
            woT_sb = big.tile([128, 4, D], bf16, tag="woT_sb")
            bias_q = big.tile([128, 4], f32, tag="bias_q")
            bias_k = big.tile([128, 4], f32, tag="bias_k")
            ident_sb = big.tile([128, 128], bf16, tag="ident_sb")
            warm = big.tile([1, 1], f32, tag="warm")
            nc.vector.memset(warm[:], 0.0)
            # prime the ACT exp table while the DMA front streams in
            nc.scalar.activation(warm[:], warm[:], AF.Exp)
            nc.vector.memset(v_sb[:, :, :, 64:65], 1.0)
            nc.sync.dma_start(bias_q[:], bq_d.rearrange("(c p) -> p c", p=128))
            nc.sync.dma_start(bias_k[:], bk_d.rearrange("(c p) -> p c", p=128))

            for _rep in range(reps):
                # ---- K projection:  k_dT[d, s] = (Wk xk^T)[d, s] + bk[d].
                # Only (st=0, all mc) runs before the attention loop; the
                # other st-chunks are spliced into the first head-pair's
                # skc loop (keys are consumed 128 at a time, so chunk st_k
                # is only needed from skc = 4*st_k onwards). ----
                wk_sb = wp.tile([128, 8, DH], bf16, tag="w", name="wk_sb")
                wr = wkT.rearrange("(kc p) m -> p kc m", p=128)
                nc.sync.dma_start(wk_sb[:], wr[:])
                xr = xkT.rearrange("(kc p) s -> p kc s", p=128)
                xt_tiles = {}

                def emit_kx_dma(st_k):
                    xt = xtp.tile([128, 8, 512], bf16, tag="xt", name=f"xt{st_k}", bufs=3)
                    for h in range(2):
                        nc.sync.dma_start(
                            xt[:, h * 4 : (h + 1) * 4],
                            xr[:, h * 4 : (h + 1) * 4, st_k * 512 : (st_k + 1) * 512],
                        )
                    xt_tiles[st_k] = xt

                emit_kx_dma(0)

                def emit_kproj(st_k, mc):
                    xt = xt_tiles[st_k]
                    ps = pop.tile([128, 512], f32, tag="T", name=f"kp{st_k}_{mc}")
                    for kc in range(8):
                        nc.tensor.matmul(
                            ps[:],
                            wk_sb[:, kc, mc * 128 : (mc + 1) * 128],
                            xt[:, kc],
                            start=(kc == 0),
                            stop=(kc == 7),
                        )
                    # per-partition bias add fused into the PSUM->SBUF
                    # copy on DVE (ACT is saturated by the exp stream)
                    nc.vector.tensor_scalar_add(
                        k_dT[:, mc, st_k * 512 : (st_k + 1) * 512],
                        ps[:],
                        bias_k[:, mc : mc + 1],
                    )

                # ---- Q projection, one Sq-tile at a time. st=0 runs up front;
                # st>0 is emitted inside the previous tile's attention loop so
                # the scheduler can drop its matmuls into PE idle gaps (ACT is
                # the pacer during attention).
                wq_sb = wp.tile([128, 8, DH], bf16, tag="w", name="wq_sb")
                wqr = wqT.rearrange("(kc p) m -> p kc m", p=128)
                nc.sync.dma_start(wq_sb[:], wqr[:])
                xq_r = xqT.rearrange("(kc p) s -> p kc s", p=128)
                qx_tiles = {}
                qp_ps = {}

                def emit_qproj(st, mc, half):
                    # half-size filler unit (~0.85us of PE) so a burst never
                    # exceeds the exp backlog ACT has banked up
                    if mc == 0 and half == 0:
                        qx = stream.tile(
                            [128, 8, 512], bf16, tag="qx", name=f"qx{st}", bufs=2
                        )
                        for kc in range(8):
                            nc.sync.dma_start(
                                qx[:, kc], xq_r[:, kc, st * 512 : (st + 1) * 512]
                            )
                        qx_tiles[st] = qx
                    qx = qx_tiles[st]
                    if half == 0:
                        qp_ps[(st, mc)] = pop.tile(
                            [128, 512], f32, tag="T", name=f"qp{st}_{mc}"
                        )
                    ps = qp_ps[(st, mc)]
                    for kc in range(half * 4, half * 4 + 4):
                        nc.tensor.matmul(
                            ps[:],
                            wq_sb[:, kc, mc * 128 : (mc + 1) * 128],
                            qx[:, kc],
                            start=(kc == 0),
                            stop=(kc == 7),
                        )
                    if half == 1:
                        nc.vector.tensor_scalar_add(
                            q_dT[:, mc, st * 512 : (st + 1) * 512],
                            ps[:],
                            bias_q[:, mc : mc + 1],
                        )

                # ---- V projection: v[sk, dv] (bf16, per-head layout). Emitted
                # as per-sk-chunk groups inside the FIRST attention head-pair's
                # loop (right before the PV matmul that consumes that chunk), so
                # ACT starts the exp stream while V is still being projected.
                wv_sb = wp.tile([128, 8, DH], bf16, tag="w", name="wv_sb")
                wvr = wvT.rearrange("(kc p) m -> p kc m", p=128)

                def emit_wv_dma():
                    nc.sync.dma_start(wv_sb[:], wvr[:])

                xv_r = xvT.rearrange("(kc p) s -> p kc s", p=128)
                vx_tiles = {}

                def emit_vx_dma(sg):
                    vx = stream.tile(
                        [128, 8, 512], bf16, tag="stream", name=f"vx{sg}", bufs=3
                    )
                    for kc in range(8):
                        nc.sync.dma_start(
                            vx[:, kc], xv_r[:, kc, sg * 512 : (sg + 1) * 512]
                        )
                    vx_tiles[sg] = vx

                def emit_vproj(skc):
                    sg, s4 = divmod(skc, 4)
                    vx = vx_tiles[sg]
                    ps = pop.tile([128, 512], f32, tag="T", name=f"vp{skc}")
                    for kc in range(8):
                        nc.tensor.matmul(
                            ps[:],
                            vx[:, kc, s4 * 128 : (s4 + 1) * 128],
                            wv_sb[:, kc],
                            start=(kc == 0),
                            stop=(kc == 7),
                        )
                    nc.vector.tensor_copy(
                        v_sb[:, skc, :, :64],
                        ps[:].rearrange("p (h d) -> p h d", h=HPC),
                    )

                # ---- output projection for one (st, nh, q4) block; emitted as
                # PE filler inside the NEXT Sq-tile's attention loop ----
                def emit_c_group(st_c, oT_tile, nh, q4, use_act=False):
                    pc = pop.tile([128, 512], f32, tag="T", name=f"pc{st_c}_{nh}_{q4}")
                    for c4 in range(4):
                        nc.tensor.matmul(
                            pc[:],
                            oT_tile[:, c4, q4 * 128 : (q4 + 1) * 128],
                            woT_sb[:, c4, nh * 512 : (nh + 1) * 512],
                            start=(c4 == 0),
                            stop=(c4 == 3),
                        )
                    ob = auxp.tile([128, 512], bf16, tag="ob", name=f"ob{st_c}_{nh}_{q4}")
                    if use_act:
                        # tail-only: ACT is idle once the exp stream has ended
                        nc.scalar.activation(ob[:], pc[:], AF.Identity)
                    else:
                        nc.vector.tensor_copy(ob[:], pc[:])
                    nc.sync.dma_start(
                        out[
                            st_c * 512 + q4 * 128 : st_c * 512 + (q4 + 1) * 128,
                            nh * 512 : (nh + 1) * 512,
                        ],
                        ob[:],
                    )

                mq = mmul.rearrange("(c p) s -> p c s", p=128)  # [128, 16, S]
                msk_tiles = {}

                def emit_msk_dma(st_m):
                    # binary mask is exact in fp8: DMA half the bytes, then
                    # upconvert to bf16 (for DVE 2x mode) on the idle Pool
                    mhal = []
                    for h2 in range(2):
                        mf = mp.tile(
                            [128, 8, 512], f8, tag="mf",
                            name=f"mskf{st_m}_{h2}", bufs=2,
                        )
                        nc.sync.dma_start(
                            mf[:],
                            mq[:, h2 * 8 : (h2 + 1) * 8,
                               st_m * 512 : (st_m + 1) * 512],
                        )
                        mt = mp.tile(
                            [128, 8, 512], bf16, tag="msk",
                            name=f"msk{st_m}_{h2}", bufs=2,
                        )
                        nc.gpsimd.tensor_copy(mt[:], mf[:])
                        mhal.append(mt)
                    msk_tiles[st_m] = mhal

                # ---- prologue: only what the first exp needs. K(st=0) and
                # Q(st=0, mc=0) run up front; K(st=1..3) and Q(st=0, mc=1..3)
                # are spliced into st0's skc loops just ahead of the
                # pipelined scores that consume them. st0's mask DMA goes
                # right behind qx0 (mask-mults gate PV in the PE FIFO). ----
                emit_kproj(0, 0)
                emit_qproj(0, 0, 0)
                emit_qproj(0, 0, 1)
                emit_msk_dma(0)
                emit_wv_dma()
                emit_vx_dma(0)
                emit_kx_dma(1)
                emit_vx_dma(1)
                emit_kx_dma(2)
                emit_vx_dma(2)
                emit_kx_dma(3)
                emit_vx_dma(3)
                # out-proj weights + transpose identity aren't needed until
                # well into the first Sq tile; keep them out of the DMA front
                if _rep == 0:
                    nc.sync.dma_start(ident_sb[:], ident_d)
                    woT_r = woT.rearrange("(c p) n -> p c n", p=128)
                    nc.sync.dma_start(woT_sb[:], woT_r[:])
                # st0 injection schedule, keyed (hp, skc): each K(st_k, mc)
                # unit lands just-in-time for the head-pair that consumes it
                # (hp h's scores need K(st_k, mc=h) by skc = 4*st_k), spreading
                # the projection mountain across all four of st0's loops
                splice = {
                    (0, 0): lambda: emit_kproj(0, 1),
                    (0, 1): lambda: emit_kproj(0, 2),
                    (0, 2): lambda: emit_kproj(0, 3),
                    (0, 3): lambda: emit_kproj(1, 0),
                    (0, 5): lambda: emit_kproj(2, 0),
                    (0, 6): lambda: emit_kproj(1, 1),
                    (0, 8): lambda: emit_kproj(3, 0),
                    (0, 9): lambda: emit_qproj(0, 1, 0),
                    (0, 12): lambda: emit_qproj(0, 1, 1),
                    (1, 0): lambda: emit_kproj(2, 1),
                    (1, 2): lambda: emit_kproj(3, 1),
                    (1, 6): lambda: emit_kproj(1, 2),
                    (1, 9): lambda: emit_qproj(0, 2, 0),
                    (1, 12): lambda: emit_qproj(0, 2, 1),
                    (2, 0): lambda: emit_kproj(2, 2),
                    (2, 2): lambda: emit_kproj(3, 2),
                    (2, 6): lambda: emit_kproj(1, 3),
                    (2, 9): lambda: emit_qproj(0, 3, 0),
                    (2, 12): lambda: emit_qproj(0, 3, 1),
                    (3, 0): lambda: emit_kproj(2, 3),
                    (3, 2): lambda: emit_kproj(3, 3),
                }

                # ---- deferred normalize+transpose for one (st, hp): o_n is
                # already written by the post-loop normalize; here we only
                # run the PE transposes + oT_sb copies (popped as filler so
                # they don't block the next head-pair's scores on PE) ----
                def emit_transpose(st_t, hp_t, o_n, oT_tile):
                    tps = pop.tile(
                        [64, 8, 128], bf16, tag="T", name=f"tp{st_t}_{hp_t}", bufs=2
                    )
                    for par in range(2):
                        for qc in range(4):
                            nc.tensor.transpose(
                                tps[:, par * 4 + qc], o_n[:, par, qc], ident_sb[:]
                            )
                        nc.vector.tensor_copy(
                            oT_tile[par * 64 : (par + 1) * 64, hp_t, :],
                            tps[:, par * 4 : (par + 1) * 4].rearrange(
                                "p a b -> p (a b)"
                            ),
                        )

                # ---- attention per 512-wide Sq tile, with next-tile q-proj,
                # previous-tile output-proj and transposes as PE filler.
                # Scores are software-pipelined ONE skc ahead: they enter the
                # PE FIFO before the current skc's PV matmuls, so the next exp
                # never waits on the exp->mask->PV chain. ----
                prev = None
                filler = []
                sthp = [(st, hp) for st in range(4) for hp in range(4)]
                ps_pend = {}

                def emit_scores(st_s, hp_s, skc_s):
                    ps_s = psp.tile(
                        [128, 1024], f32, tag="ps", name=f"ps{st_s}_{hp_s}_{skc_s}"
                    )
                    sk = slice(skc_s * 128, (skc_s + 1) * 128)
                    sq_s = slice(st_s * 512, (st_s + 1) * 512)
                    for par in range(2):
                        b0 = par * 64
                        nc.tensor.matmul(
                            ps_s[:, par * 512 : (par + 1) * 512],
                            k_dT[b0 : b0 + 64, hp_s, sk],
                            q_dT[b0 : b0 + 64, hp_s, sq_s],
                            start=True,
                            stop=True,
                        )
                    ps_pend[(st_s, hp_s, skc_s)] = ps_s

                oT_tiles = {}
                for idx, (st, hp) in enumerate(sthp):
                    if hp == 0:
                        if st not in msk_tiles:
                            emit_msk_dma(st)
                        oT_tiles[st] = otp.tile(
                            [128, 4, 512], bf16, tag="oT_sb", name=f"oT{st}"
                        )
                    mskh = msk_tiles[st]
                    oT_sb = oT_tiles[st]
                    # o accumulators: [q=128, qc=4, 65] per head; column 64
                    # collects the softmax denominator via v_aug's ones col
                    o_ps = [
                        pop.tile(
                            [128, 4, 65], f32, tag="o",
                            name=f"o{st}_{hp}_{i}", bufs=2,
                        )
                        for i in range(2)
                    ]
                    if idx == 0:
                        emit_scores(st, hp, 0)
                    def emit_pv(s_pv, exs):
                        # PV with P as stationary (128-wide: full PE cols).
                        # o_ps[par][:, qc] += ex_chunk.T @ v_aug
                        for par in range(2):
                            for qc in range(4):
                                # start only on the tile's first group: start
                                # clears has_written for the WHOLE bank, so a
                                # per-group start would wipe the accumulation
                                # state of groups started earlier in the bank
                                nc.tensor.matmul(
                                    o_ps[par][:, qc],
                                    exs[:, par * 512 + qc * 128 : par * 512 + (qc + 1) * 128],
                                    v_sb[:, s_pv, hp * 2 + par],
                                    start=(s_pv == 0 and qc == 0),
                                    stop=(s_pv == 15 and qc == 3),
                                    skip_group_check=True,
                                )

                    ex_map = {}
                    for skc in range(16):
                        if st == 0 and hp == 0:
                            emit_vproj(skc)
                        ps_s = ps_pend.pop((st, hp, skc))
                        ex = small.tile([128, 1024], bf16, tag="ex", bufs=8)
                        nc.scalar.activation(ex[:], ps_s[:], AF.Exp)
                        # binary mask applied multiplicatively, in place
                        nc.vector.tensor_mul(
                            ex.rearrange("p (t s) -> p t s", t=2),
                            ex.rearrange("p (t s) -> p t s", t=2),
                            mskh[skc // 8][:, skc % 8, None, :].to_broadcast((128, 2, 512)),
                        )
                        ex_map[skc] = ex
                        # spliced K/Q projection units (st0 only), placed at
                        # their LATEST-safe slot so a DMA-gated unit never
                        # parks in the PE FIFO ahead of runnable scores
                        if st == 0 and (hp, skc) in splice:
                            splice[(hp, skc)]()
                        # next scores enter the PE FIFO before this skc's PV
                        if skc < 15:
                            emit_scores(st, hp, skc + 1)
                        elif idx + 1 < len(sthp):
                            nst, nhp = sthp[idx + 1]
                            emit_scores(nst, nhp, 0)
                        if skc % 4 == 2 and filler:
                            filler.pop(0)()
                        if st == 0 and hp == 0:
                            if skc >= 2:
                                emit_pv(skc - 2, ex_map.pop(skc - 2))
                        else:
                            emit_pv(skc, ex_map.pop(skc))
                    if st == 0 and hp == 0:
                        for s_tail in (14, 15):
                            emit_pv(s_tail, ex_map.pop(s_tail))
                    # normalize: per-q reciprocal of the denominator column,
                    # then per-partition scalar multiply (q is on partitions)
                    o_n = small.tile(
                        [128, 2, 4, 64], bf16, tag="on", name=f"on{st}_{hp}", bufs=2
                    )
                    for par in range(2):
                        rc = auxp.tile([128, 4, 1], f32, tag="rc")
                        nc.vector.reciprocal(rc[:], o_ps[par][:, :, 64:65])
                        nc.vector.tensor_mul(
                            o_n[:, par],
                            o_ps[par][:, :, 0:64],
                            rc.to_broadcast((128, 4, 64)),
                        )
                    filler.append(
                        lambda st=st, hp=hp, o_n=o_n, oT=oT_sb: emit_transpose(
                            st, hp, o_n, oT
                        )
                    )
                    # queue PE filler work; it is popped mid-skc-loop of the
                    # following head pair (ACT-paced steady state has ~40%
                    # PE idle to absorb it)
                    if st < 3:
                        filler.append(lambda st=st, hp=hp: emit_qproj(st + 1, hp, 0))
                        filler.append(lambda st=st, hp=hp: emit_qproj(st + 1, hp, 1))
                    if prev is not None:
                        for j in (2 * hp, 2 * hp + 1):
                            filler.append(
                                lambda p=prev, j=j: emit_c_group(
                                    p[0], p[1], j // 4, j % 4
                                )
                            )
                    if hp == 3:
                        prev = (st, oT_sb)
                while filler:
                    filler.pop(0)()
                for j in range(8):
                    emit_c_group(3, prev[1], j // 4, j % 4, use_act=True)

    nc.compile()
    _CACHE[reps] = nc
    return nc


def _prepare_in_maps(q_in, k_in, v_in, m_in, Wq, bq, Wk, bk, Wv, Wo):
    import ml_dtypes

    bf16 = ml_dtypes.bfloat16
    f8 = ml_dtypes.float8_e4m3
    f32 = np.float32

    ident = np.eye(128, dtype=np.float32).astype(bf16)
    per_half = []
    for hh in range(2):
        sl = slice(hh * DH, (hh + 1) * DH)
        per_half.append(
            dict(
                wqT=np.ascontiguousarray((Wq[sl, :] / 8.0).T, f32).astype(bf16),
                wkT=np.ascontiguousarray(Wk[sl, :].T, f32).astype(bf16),
                wvT=np.ascontiguousarray(Wv[sl, :].T, f32).astype(bf16),
                woT=np.ascontiguousarray(Wo[:, sl].T, f32).astype(bf16),
                bq=np.ascontiguousarray(bq[sl] / 8.0, f32),
                bk=np.ascontiguousarray(bk[sl], f32),
                ident=ident,
            )
        )

    in_maps = []
    for b in range(4):
        xqT = np.ascontiguousarray(q_in[b].T, f32).astype(bf16)
        xkT = np.ascontiguousarray(k_in[b].T, f32).astype(bf16)
        xvT = np.ascontiguousarray(v_in[b].T, f32).astype(bf16)
        maskmul = np.ascontiguousarray((1.0 - m_in[b, 0].T)).astype(f8)
        for hh in range(2):
            m = dict(xqT=xqT, xkT=xkT, xvT=xvT, maskmul=maskmul)
            m.update(per_half[hh])
            in_maps.append(m)
    return in_maps


def _run(inputs, trace=False, trace_kwargs=None):
    from concourse import bass_utils

    q_in = np.asarray(inputs["q_in"], np.float32)
    k_in = np.asarray(inputs["k_in"], np.float32)
    v_in = np.asarray(inputs["v_in"], np.float32)
    m_in = np.asarray(inputs["m_in"], np.float32)
    Wq = np.asarray(inputs["Wq"], np.float32)
    bq = np.asarray(inputs["bq"], np.float32)
    Wk = np.asarray(inputs["Wk"], np.float32)
    bk = np.asarray(inputs["bk"], np.float32)
    Wv = np.asarray(inputs["Wv"], np.float32)
    bv = np.asarray(inputs["bv"], np.float32)
    Wo = np.asarray(inputs["Wo"], np.float32)
    bo = np.asarray(inputs["bo"], np.float32)

    nc = _build_program()
    in_maps = _prepare_in_maps(q_in, k_in, v_in, m_in, Wq, bq, Wk, bk, Wv, Wo)
    kw = {}
    if trace:
        kw["trace"] = True
        if trace_kwargs:
            kw["trace_kwargs"] = trace_kwargs
    res = bass_utils.run_bass_kernel_spmd(
        nc, in_maps, core_ids=list(range(N_CORES)), **kw
    )

    total_bias = (bo + bv @ Wo.T).astype(np.float32)
    output = np.empty((4, S, D), np.float32)
    for b in range(4):
        output[b] = res.results[2 * b]["out"].astype(np.float32)
        output[b] += res.results[2 * b + 1]["out"].astype(np.float32)
        output[b] += total_bias
    return output, res


def kernel(**inputs) -> np.ndarray:
    output, _ = _run(inputs, trace=False)
    return output


def run_traced(inputs):
    """For test.py: returns (output, BassKernelResults with exec_time_ns)."""
    return _run(inputs, trace=True)



# revision 6
# speedup vs baseline: 1.2060x; 1.2060x over previous
"""Multi-head attention (B=4, S=2048, D=1024, H=16) on 8 trn2 NeuronCores.

Sharding: batch x head-half. Core c handles batch b = c//2 and heads
hh*8..hh*8+8 where hh = c%2. Each core computes its heads' Q/K/V
projections, attention, and a partial output projection; the host sums
the two partials per batch and adds the (constant) bias terms.

Device-side layout choices (per core, S=2048, DH=512 head dims):
  q_dT, k_dT : [depth-dims, S]  (f32r, matmul-ready: contraction on partitions)
  scores_T   : [Sk, Sq] tiles  = k_dT.T @ q_dT  (K=64, head pairs packed into
               PE row-groups 0-1 / 2-3; pipelined one skc ahead of exp)
  attn       : exp on ACT (PSUM->SBUF, bf16), multiplicative binary mask on DVE
  PV swap    : o[q, d] += P_chunk.T @ v_aug with P (the attn weights) as the
               128-wide stationary operand — halves PE rows vs stationary-V.
               v_aug col 64 is all-ones so o[:, 64] is the softmax denominator.
  normalize  : per-partition reciprocal + scalar-broadcast multiply on DVE
               (q sits on partitions, so no cross-partition broadcast needed)
  transpose  : o_n[q, d] -> oT[d, q] via PE transpose-mode (identity operand),
               deferred into the next head-pair's loop as PE filler
  out proj   : out[Sq, 1024] = o_T.T @ WoT  (bf16), DMA to DRAM
  x/w DMA    : inputs and projection weights staged as bf16 (halves HBM
               traffic); K(st>0) / Q(st0, mc>0) projections are spliced
               just-in-time into st0's four head-pair loops so the exp
               stream starts ~26us in instead of ~66us

Scale 1/sqrt(depth) is folded into Wq/bq on the host. bv and bo are folded
into a constant host-side bias (attention rows sum to 1).
"""

import numpy as np

D = 1024
S = 2048
HPC = 8          # heads per core
DH = HPC * 64    # 512 per-core head dims
N_CORES = 8

_CACHE = {}

# blob layout (bf16 element offsets); assembled by _prepare_in_maps
OFF_XQ = 0
OFF_XK = D * S                      # 2097152
OFF_XV = 2 * D * S
OFF_WQ = 3 * D * S
OFF_WK = OFF_WQ + D * DH
OFF_WV = OFF_WK + D * DH
OFF_WO = OFF_WV + D * DH
OFF_ID = OFF_WO + DH * D
OFF_BQ = OFF_ID + 128 * 128
OFF_BK = OFF_BQ + DH
OFF_MM = OFF_BK + DH                # S*S fp8 bytes as S*S/2 bf16 slots
BLOB_TOT = OFF_MM + S * S // 2


def _build_program(reps=1):
    if reps in _CACHE:
        return _CACHE[reps]

    from concourse import bacc, tile, mybir

    f32 = mybir.dt.float32
    f32r = mybir.dt.float32r
    bf16 = mybir.dt.bfloat16
    f8 = mybir.dt.float8e4
    AF = mybir.ActivationFunctionType

    nc = bacc.Bacc(
        "TRN2",
        target_bir_lowering=False,
        debug=False,
        enable_asserts=False,
        num_devices=N_CORES,
    )

    # All inputs packed into ONE dram blob: per-execution input staging has
    # a large per-buffer fixed cost (~56us each through this PJRT path), so
    # 11 separate inputs cost ~600us before the kernel even starts. The
    # mask's fp8 bytes live in the blob bitcast as bf16 pairs; biases are
    # stored bf16 (|b|~0.01, rounding negligible vs the bf16 matmul noise).
    blob = nc.dram_tensor("blob", [BLOB_TOT], bf16, kind="ExternalInput").ap()
    out = nc.dram_tensor("out", [S, D], bf16, kind="ExternalOutput").ap()

    def seg(off, n):
        return blob[off : off + n]

    xqT = seg(OFF_XQ, D * S)      # flat, row-major [D, S]
    xkT = seg(OFF_XK, D * S)
    xvT = seg(OFF_XV, D * S)
    wqT = seg(OFF_WQ, D * DH)     # flat, row-major [D, DH]
    wkT = seg(OFF_WK, D * DH)
    wvT = seg(OFF_WV, D * DH)
    woT = seg(OFF_WO, DH * D)     # flat, row-major [DH, D]
    ident_d = seg(OFF_ID, 128 * 128).rearrange("(p n) -> p n", p=128)
    bq_d = seg(OFF_BQ, DH)        # bf16
    bk_d = seg(OFF_BK, DH)        # bf16
    mmul = seg(OFF_MM, S * S // 2).bitcast(f8)  # flat f8, row-major [S, S]

    with tile.TileContext(nc) as tc:
        with (
            nc.allow_low_precision(reason="f32r is fp32-width; rounding intended"),
            tc.tile_pool(name="big", bufs=1) as big,
            tc.tile_pool(name="ot", bufs=2) as otp,
            tc.tile_pool(name="wp", bufs=3) as wp,
            tc.tile_pool(name="stream", bufs=2) as stream,
            tc.tile_pool(name="xtp", bufs=2) as xtp,
            tc.tile_pool(name="mp", bufs=2) as mp,
            tc.tile_pool(name="small", bufs=3) as small,
            tc.tile_pool(name="aux", bufs=4) as auxp,
            tc.tile_pool(name="ps", bufs=2, space="PSUM") as psp,
            tc.tile_pool(name="po", bufs=2, space="PSUM") as pop,
        ):
            # ---- persistent tiles ----
            q_dT = big.tile([128, 4, S], bf16, tag="q_dT")
            k_dT = big.tile([128, 4, S], bf16, tag="k_dT")
            v_sb = big.tile([128, 16, HPC, 65], bf16, tag="v_sb")
            woT_sb = big.tile([128, 4, D], bf16, tag="woT_sb")
            bias_q = big.tile([128, 4], f32, tag="bias_q")
            bias_k = big.tile([128, 4], f32, tag="bias_k")
            ident_sb = big.tile([128, 128], bf16, tag="ident_sb")
            warm = big.tile([1, 1], f32, tag="warm")
            nc.vector.memset(warm[:], 0.0)
            # prime the ACT exp table while the DMA front streams in
            nc.scalar.activation(warm[:], warm[:], AF.Exp)
            nc.vector.memset(v_sb[:, :, :, 64:65], 1.0)
            # biases arrive bf16 in the blob; convert to f32 on idle Pool
            bias_qh = big.tile([128, 4], bf16, tag="bias_qh")
            bias_kh = big.tile([128, 4], bf16, tag="bias_kh")
            nc.sync.dma_start(bias_qh[:], bq_d.rearrange("(c p) -> p c", p=128))
            nc.sync.dma_start(bias_kh[:], bk_d.rearrange("(c p) -> p c", p=128))
            nc.gpsimd.tensor_copy(bias_q[:], bias_qh[:])
            nc.gpsimd.tensor_copy(bias_k[:], bias_kh[:])

            for _rep in range(reps):
                # ---- K projection:  k_dT[d, s] = (Wk xk^T)[d, s] + bk[d].
                # Only (st=0, all mc) runs before the attention loop; the
                # other st-chunks are spliced into the first head-pair's
                # skc loop (keys are consumed 128 at a time, so chunk st_k
                # is only needed from skc = 4*st_k onwards). ----
                wk_sb = wp.tile([128, 8, DH], bf16, tag="w", name="wk_sb")
                wr = wkT.rearrange("(kc p m) -> p kc m", p=128, m=DH)
                nc.sync.dma_start(wk_sb[:], wr[:])
                xr = xkT.rearrange("(kc p s) -> p kc s", p=128, s=S)
                xt_tiles = {}

                def emit_kx_dma(st_k):
                    xt = xtp.tile([128, 8, 512], bf16, tag="xt", name=f"xt{st_k}", bufs=3)
                    for h in range(2):
                        nc.sync.dma_start(
                            xt[:, h * 4 : (h + 1) * 4],
                            xr[:, h * 4 : (h + 1) * 4, st_k * 512 : (st_k + 1) * 512],
                        )
                    xt_tiles[st_k] = xt

                emit_kx_dma(0)

                def emit_kproj(st_k, mc):
                    xt = xt_tiles[st_k]
                    ps = pop.tile([128, 512], f32, tag="T", name=f"kp{st_k}_{mc}")
                    for kc in range(8):
                        nc.tensor.matmul(
                            ps[:],
                            wk_sb[:, kc, mc * 128 : (mc + 1) * 128],
                            xt[:, kc],
                            start=(kc == 0),
                            stop=(kc == 7),
                        )
                    # per-partition bias add fused into the PSUM->SBUF
                    # copy on DVE (ACT is saturated by the exp stream)
                    nc.vector.tensor_scalar_add(
                        k_dT[:, mc, st_k * 512 : (st_k + 1) * 512],
                        ps[:],
                        bias_k[:, mc : mc + 1],
                    )

                # ---- Q projection, one Sq-tile at a time. st=0 runs up front;
                # st>0 is emitted inside the previous tile's attention loop so
                # the scheduler can drop its matmuls into PE idle gaps (ACT is
                # the pacer during attention).
                wq_sb = wp.tile([128, 8, DH], bf16, tag="w", name="wq_sb")
                wqr = wqT.rearrange("(kc p m) -> p kc m", p=128, m=DH)
                nc.sync.dma_start(wq_sb[:], wqr[:])
                xq_r = xqT.rearrange("(kc p s) -> p kc s", p=128, s=S)
                qx_tiles = {}
                qp_ps = {}

                def emit_qproj(st, mc, half):
                    # half-size filler unit (~0.85us of PE) so a burst never
                    # exceeds the exp backlog ACT has banked up
                    if mc == 0 and half == 0:
                        qx = stream.tile(
                            [128, 8, 512], bf16, tag="qx", name=f"qx{st}", bufs=2
                        )
                        for kc in range(8):
                            nc.sync.dma_start(
                                qx[:, kc], xq_r[:, kc, st * 512 : (st + 1) * 512]
                            )
                        qx_tiles[st] = qx
                    qx = qx_tiles[st]
                    if half == 0:
                        qp_ps[(st, mc)] = pop.tile(
                            [128, 512], f32, tag="T", name=f"qp{st}_{mc}"
                        )
                    ps = qp_ps[(st, mc)]
                    for kc in range(half * 4, half * 4 + 4):
                        nc.tensor.matmul(
                            ps[:],
                            wq_sb[:, kc, mc * 128 : (mc + 1) * 128],
                            qx[:, kc],
                            start=(kc == 0),
                            stop=(kc == 7),
                        )
                    if half == 1:
                        nc.vector.tensor_scalar_add(
                            q_dT[:, mc, st * 512 : (st + 1) * 512],
                            ps[:],
                            bias_q[:, mc : mc + 1],
                        )

                # ---- V projection: v[sk, dv] (bf16, per-head layout). Emitted
                # as per-sk-chunk groups inside the FIRST attention head-pair's
                # loop (right before the PV matmul that consumes that chunk), so
                # ACT starts the exp stream while V is still being projected.
                wv_sb = wp.tile([128, 8, DH], bf16, tag="w", name="wv_sb")
                wvr = wvT.rearrange("(kc p m) -> p kc m", p=128, m=DH)

                def emit_wv_dma():
                    nc.sync.dma_start(wv_sb[:], wvr[:])

                xv_r = xvT.rearrange("(kc p s) -> p kc s", p=128, s=S)
                vx_tiles = {}

                def emit_vx_dma(sg):
                    vx = stream.tile(
                        [128, 8, 512], bf16, tag="stream", name=f"vx{sg}", bufs=3
                    )
                    for kc in range(8):
                        nc.sync.dma_start(
                            vx[:, kc], xv_r[:, kc, sg * 512 : (sg + 1) * 512]
                        )
                    vx_tiles[sg] = vx

                def emit_vproj(skc):
                    sg, s4 = divmod(skc, 4)
                    vx = vx_tiles[sg]
                    ps = pop.tile([128, 512], f32, tag="T", name=f"vp{skc}")
                    for kc in range(8):
                        nc.tensor.matmul(
                            ps[:],
                            vx[:, kc, s4 * 128 : (s4 + 1) * 128],
                            wv_sb[:, kc],
                            start=(kc == 0),
                            stop=(kc == 7),
                        )
                    nc.vector.tensor_copy(
                        v_sb[:, skc, :, :64],
                        ps[:].rearrange("p (h d) -> p h d", h=HPC),
                    )

                # ---- output projection for one (st, nh, q4) block; emitted as
                # PE filler inside the NEXT Sq-tile's attention loop ----
                def emit_c_group(st_c, oT_tile, nh, q4, use_act=False):
                    pc = pop.tile([128, 512], f32, tag="T", name=f"pc{st_c}_{nh}_{q4}")
                    for c4 in range(4):
                        nc.tensor.matmul(
                            pc[:],
                            oT_tile[:, c4, q4 * 128 : (q4 + 1) * 128],
                            woT_sb[:, c4, nh * 512 : (nh + 1) * 512],
                            start=(c4 == 0),
                            stop=(c4 == 3),
                        )
                    ob = auxp.tile([128, 512], bf16, tag="ob", name=f"ob{st_c}_{nh}_{q4}")
                    if use_act:
                        # tail-only: ACT is idle once the exp stream has ended
                        nc.scalar.activation(ob[:], pc[:], AF.Identity)
                    else:
                        nc.vector.tensor_copy(ob[:], pc[:])
                    nc.sync.dma_start(
                        out[
                            st_c * 512 + q4 * 128 : st_c * 512 + (q4 + 1) * 128,
                            nh * 512 : (nh + 1) * 512,
                        ],
                        ob[:],
                    )

                mq = mmul.rearrange("(c p s) -> p c s", p=128, s=S)  # [128, 16, S]
                msk_tiles = {}

                def emit_msk_dma(st_m):
                    # binary mask is exact in fp8: DMA half the bytes, then
                    # upconvert to bf16 (for DVE 2x mode) on the idle Pool
                    mhal = []
                    for h2 in range(2):
                        mf = mp.tile(
                            [128, 8, 512], f8, tag="mf",
                            name=f"mskf{st_m}_{h2}", bufs=2,
                        )
                        nc.sync.dma_start(
                            mf[:],
                            mq[:, h2 * 8 : (h2 + 1) * 8,
                               st_m * 512 : (st_m + 1) * 512],
                        )
                        mt = mp.tile(
                            [128, 8, 512], bf16, tag="msk",
                            name=f"msk{st_m}_{h2}", bufs=2,
                        )
                        nc.gpsimd.tensor_copy(mt[:], mf[:])
                        mhal.append(mt)
                    msk_tiles[st_m] = mhal

                # ---- prologue: only what the first exp needs. K(st=0) and
                # Q(st=0, mc=0) run up front; K(st=1..3) and Q(st=0, mc=1..3)
                # are spliced into st0's skc loops just ahead of the
                # pipelined scores that consume them. st0's mask DMA goes
                # right behind qx0 (mask-mults gate PV in the PE FIFO). ----
                emit_kproj(0, 0)
                emit_qproj(0, 0, 0)
                emit_qproj(0, 0, 1)
                emit_msk_dma(0)
                emit_wv_dma()
                emit_vx_dma(0)
                emit_kx_dma(1)
                emit_vx_dma(1)
                emit_kx_dma(2)
                emit_vx_dma(2)
                emit_kx_dma(3)
                emit_vx_dma(3)
                # out-proj weights + transpose identity aren't needed until
                # well into the first Sq tile; keep them out of the DMA front
                if _rep == 0:
                    nc.sync.dma_start(ident_sb[:], ident_d)
                    woT_r = woT.rearrange("(c p n) -> p c n", p=128, n=D)
                    nc.sync.dma_start(woT_sb[:], woT_r[:])
                # st0 injection schedule, keyed (hp, skc): each K(st_k, mc)
                # unit lands just-in-time for the head-pair that consumes it
                # (hp h's scores need K(st_k, mc=h) by skc = 4*st_k), spreading
                # the projection mountain across all four of st0's loops
                splice = {
                    (0, 0): lambda: emit_kproj(0, 1),
                    (0, 1): lambda: emit_kproj(0, 2),
                    (0, 2): lambda: emit_kproj(0, 3),
                    (0, 3): lambda: emit_kproj(1, 0),
                    (0, 5): lambda: emit_kproj(2, 0),
                    (0, 6): lambda: emit_kproj(1, 1),
                    (0, 8): lambda: emit_kproj(3, 0),
                    (0, 9): lambda: emit_qproj(0, 1, 0),
                    (0, 12): lambda: emit_qproj(0, 1, 1),
                    (1, 0): lambda: emit_kproj(2, 1),
                    (1, 2): lambda: emit_kproj(3, 1),
                    (1, 6): lambda: emit_kproj(1, 2),
                    (1, 9): lambda: emit_qproj(0, 2, 0),
                    (1, 12): lambda: emit_qproj(0, 2, 1),
                    (2, 0): lambda: emit_kproj(2, 2),
                    (2, 2): lambda: emit_kproj(3, 2),
                    (2, 6): lambda: emit_kproj(1, 3),
                    (2, 9): lambda: emit_qproj(0, 3, 0),
                    (2, 12): lambda: emit_qproj(0, 3, 1),
                    (3, 0): lambda: emit_kproj(2, 3),
                    (3, 2): lambda: emit_kproj(3, 3),
                }

                # ---- deferred normalize+transpose for one (st, hp): o_n is
                # already written by the post-loop normalize; here we only
                # run the PE transposes + oT_sb copies (popped as filler so
                # they don't block the next head-pair's scores on PE) ----
                def emit_transpose(st_t, hp_t, o_n, oT_tile):
                    tps = pop.tile(
                        [64, 8, 128], bf16, tag="T", name=f"tp{st_t}_{hp_t}", bufs=2
                    )
                    for par in range(2):
                        for qc in range(4):
                            nc.tensor.transpose(
                                tps[:, par * 4 + qc], o_n[:, par, qc], ident_sb[:]
                            )
                        nc.vector.tensor_copy(
                            oT_tile[par * 64 : (par + 1) * 64, hp_t, :],
                            tps[:, par * 4 : (par + 1) * 4].rearrange(
                                "p a b -> p (a b)"
                            ),
                        )

                # ---- attention per 512-wide Sq tile, with next-tile q-proj,
                # previous-tile output-proj and transposes as PE filler.
                # Scores are software-pipelined ONE skc ahead: they enter the
                # PE FIFO before the current skc's PV matmuls, so the next exp
                # never waits on the exp->mask->PV chain. ----
                prev = None
                filler = []
                sthp = [(st, hp) for st in range(4) for hp in range(4)]
                ps_pend = {}

                def emit_scores(st_s, hp_s, skc_s):
                    ps_s = psp.tile(
                        [128, 1024], f32, tag="ps", name=f"ps{st_s}_{hp_s}_{skc_s}"
                    )
                    sk = slice(skc_s * 128, (skc_s + 1) * 128)
                    sq_s = slice(st_s * 512, (st_s + 1) * 512)
                    for par in range(2):
                        b0 = par * 64
                        nc.tensor.matmul(
                            ps_s[:, par * 512 : (par + 1) * 512],
                            k_dT[b0 : b0 + 64, hp_s, sk],
                            q_dT[b0 : b0 + 64, hp_s, sq_s],
                            start=True,
                            stop=True,
                        )
                    ps_pend[(st_s, hp_s, skc_s)] = ps_s

                oT_tiles = {}
                for idx, (st, hp) in enumerate(sthp):
                    if hp == 0:
                        if st not in msk_tiles:
                            emit_msk_dma(st)
                        oT_tiles[st] = otp.tile(
                            [128, 4, 512], bf16, tag="oT_sb", name=f"oT{st}"
                        )
                    mskh = msk_tiles[st]
                    oT_sb = oT_tiles[st]
                    # o accumulators: [q=128, qc=4, 65] per head; column 64
                    # collects the softmax denominator via v_aug's ones col
                    o_ps = [
                        pop.tile(
                            [128, 4, 65], f32, tag="o",
                            name=f"o{st}_{hp}_{i}", bufs=2,
                        )
                        for i in range(2)
                    ]
                    if idx == 0:
                        emit_scores(st, hp, 0)
                    def emit_pv(s_pv, exs):
                        # PV with P as stationary (128-wide: full PE cols).
                        # o_ps[par][:, qc] += ex_chunk.T @ v_aug
                        for par in range(2):
                            for qc in range(4):
                                # start only on the tile's first group: start
                                # clears has_written for the WHOLE bank, so a
                                # per-group start would wipe the accumulation
                                # state of groups started earlier in the bank
                                nc.tensor.matmul(
                                    o_ps[par][:, qc],
                                    exs[:, par * 512 + qc * 128 : par * 512 + (qc + 1) * 128],
                                    v_sb[:, s_pv, hp * 2 + par],
                                    start=(s_pv == 0 and qc == 0),
                                    stop=(s_pv == 15 and qc == 3),
                                    skip_group_check=True,
                                )

                    ex_map = {}
                    for skc in range(16):
                        if st == 0 and hp == 0:
                            emit_vproj(skc)
                        ps_s = ps_pend.pop((st, hp, skc))
                        ex = small.tile([128, 1024], bf16, tag="ex", bufs=8)
                        nc.scalar.activation(ex[:], ps_s[:], AF.Exp)
                        # binary mask applied multiplicatively, in place
                        nc.vector.tensor_mul(
                            ex.rearrange("p (t s) -> p t s", t=2),
                            ex.rearrange("p (t s) -> p t s", t=2),
                            mskh[skc // 8][:, skc % 8, None, :].to_broadcast((128, 2, 512)),
                        )
                        ex_map[skc] = ex
                        # spliced K/Q projection units (st0 only), placed at
                        # their LATEST-safe slot so a DMA-gated unit never
                        # parks in the PE FIFO ahead of runnable scores
                        if st == 0 and (hp, skc) in splice:
                            splice[(hp, skc)]()
                        # next scores enter the PE FIFO before this skc's PV
                        if skc < 15:
                            emit_scores(st, hp, skc + 1)
                        elif idx + 1 < len(sthp):
                            nst, nhp = sthp[idx + 1]
                            emit_scores(nst, nhp, 0)
                        if skc % 4 == 2 and filler:
                            filler.pop(0)()
                        if st == 0 and hp == 0:
                            if skc >= 2:
                                emit_pv(skc - 2, ex_map.pop(skc - 2))
                        else:
                            emit_pv(skc, ex_map.pop(skc))
                    if st == 0 and hp == 0:
                        for s_tail in (14, 15):
                            emit_pv(s_tail, ex_map.pop(s_tail))
                    # normalize: per-q reciprocal of the denominator column,
                    # then per-partition scalar multiply (q is on partitions)
                    o_n = small.tile(
                        [128, 2, 4, 64], bf16, tag="on", name=f"on{st}_{hp}", bufs=2
                    )
                    for par in range(2):
                        rc = auxp.tile([128, 4, 1], f32, tag="rc")
                        nc.vector.reciprocal(rc[:], o_ps[par][:, :, 64:65])
                        nc.vector.tensor_mul(
                            o_n[:, par],
                            o_ps[par][:, :, 0:64],
                            rc.to_broadcast((128, 4, 64)),
                        )
                    filler.append(
                        lambda st=st, hp=hp, o_n=o_n, oT=oT_sb: emit_transpose(
                            st, hp, o_n, oT
                        )
                    )
                    # queue PE filler work; it is popped mid-skc-loop of the
                    # following head pair (ACT-paced steady state has ~40%
                    # PE idle to absorb it)
                    if st < 3:
                        filler.append(lambda st=st, hp=hp: emit_qproj(st + 1, hp, 0))
                        filler.append(lambda st=st, hp=hp: emit_qproj(st + 1, hp, 1))
                    if prev is not None:
                        for j in (2 * hp, 2 * hp + 1):
                            filler.append(
                                lambda p=prev, j=j: emit_c_group(
                                    p[0], p[1], j // 4, j % 4
                                )
                            )
                    if hp == 3:
                        prev = (st, oT_sb)
                while filler:
                    filler.pop(0)()
                for j in range(8):
                    emit_c_group(3, prev[1], j // 4, j % 4, use_act=True)

    nc.compile()
    _CACHE[reps] = nc
    return nc


def _prepare_in_maps(q_in, k_in, v_in, m_in, Wq, bq, Wk, bk, Wv, Wo):
    import ml_dtypes

    bf16 = ml_dtypes.bfloat16
    f8 = ml_dtypes.float8_e4m3
    f32 = np.float32

    ident = np.eye(128, dtype=np.float32).astype(bf16).reshape(-1)
    per_half = []
    for hh in range(2):
        sl = slice(hh * DH, (hh + 1) * DH)
        per_half.append(
            [
                np.ascontiguousarray((Wq[sl, :] / 8.0).T, f32).astype(bf16).reshape(-1),
                np.ascontiguousarray(Wk[sl, :].T, f32).astype(bf16).reshape(-1),
                np.ascontiguousarray(Wv[sl, :].T, f32).astype(bf16).reshape(-1),
                np.ascontiguousarray(Wo[:, sl].T, f32).astype(bf16).reshape(-1),
                ident,
                np.ascontiguousarray(bq[sl] / 8.0, f32).astype(bf16),
                np.ascontiguousarray(bk[sl], f32).astype(bf16),
            ]
        )

    in_maps = []
    for b in range(4):
        xqT = np.ascontiguousarray(q_in[b].T, f32).astype(bf16).reshape(-1)
        xkT = np.ascontiguousarray(k_in[b].T, f32).astype(bf16).reshape(-1)
        xvT = np.ascontiguousarray(v_in[b].T, f32).astype(bf16).reshape(-1)
        maskmul = np.ascontiguousarray((1.0 - m_in[b, 0].T)).astype(f8)
        mask_bf = maskmul.reshape(-1).view(bf16)
        for hh in range(2):
            blob = np.concatenate(
                [xqT, xkT, xvT] + per_half[hh] + [mask_bf]
            )
            assert blob.size == BLOB_TOT
            in_maps.append({"blob": blob})
    return in_maps


def _run(inputs, trace=False, trace_kwargs=None):
    from concourse import bass_utils

    q_in = np.asarray(inputs["q_in"], np.float32)
    k_in = np.asarray(inputs["k_in"], np.float32)
    v_in = np.asarray(inputs["v_in"], np.float32)
    m_in = np.asarray(inputs["m_in"], np.float32)
    Wq = np.asarray(inputs["Wq"], np.float32)
    bq = np.asarray(inputs["bq"], np.float32)
    Wk = np.asarray(inputs["Wk"], np.float32)
    bk = np.asarray(inputs["bk"], np.float32)
    Wv = np.asarray(inputs["Wv"], np.float32)
    bv = np.asarray(inputs["bv"], np.float32)
    Wo = np.asarray(inputs["Wo"], np.float32)
    bo = np.asarray(inputs["bo"], np.float32)

    nc = _build_program()
    in_maps = _prepare_in_maps(q_in, k_in, v_in, m_in, Wq, bq, Wk, bk, Wv, Wo)
    kw = {}
    if trace:
        kw["trace"] = True
        if trace_kwargs:
            kw["trace_kwargs"] = trace_kwargs
    res = bass_utils.run_bass_kernel_spmd(
        nc, in_maps, core_ids=list(range(N_CORES)), **kw
    )

    total_bias = (bo + bv @ Wo.T).astype(np.float32)
    output = np.empty((4, S, D), np.float32)
    for b in range(4):
        output[b] = res.results[2 * b]["out"].astype(np.float32)
        output[b] += res.results[2 * b + 1]["out"].astype(np.float32)
        output[b] += total_bias
    return output, res


def kernel(**inputs) -> np.ndarray:
    output, _ = _run(inputs, trace=False)
    return output


def run_traced(inputs):
    """For test.py: returns (output, BassKernelResults with exec_time_ns)."""
    return _run(inputs, trace=True)



# revision 19
# speedup vs baseline: 2.3897x; 1.9815x over previous
"""Multi-head attention (B=4, S=2048, D=1024, H=16) on 8 trn2 NeuronCores.

Sharding: batch x head-half. Core c handles batch b = c//2 and heads
hh*8..hh*8+8 where hh = c%2. Each core computes its heads' Q/K/V
projections, attention, and a partial output projection; the host sums
the two partials per batch and adds the (constant) bias terms.

Device-side layout choices (per core, S=2048, DH=512 head dims):
  q_dT, k_dT : [depth-dims, S]  (f32r, matmul-ready: contraction on partitions)
  scores_T   : [Sk, Sq] tiles  = k_dT.T @ q_dT  (K=64, head pairs packed into
               PE row-groups 0-1 / 2-3; pipelined one skc ahead of exp)
  attn       : exp on ACT (PSUM->SBUF, bf16), multiplicative binary mask on DVE
  PV swap    : o[q, d] += P_chunk.T @ v_aug with P (the attn weights) as the
               128-wide stationary operand — halves PE rows vs stationary-V.
               v_aug col 64 is all-ones so o[:, 64] is the softmax denominator.
  normalize  : per-partition reciprocal + scalar-broadcast multiply on DVE
               (q sits on partitions, so no cross-partition broadcast needed)
  transpose  : o_n[q, d] -> oT[d, q] via PE transpose-mode (identity operand),
               deferred into the next head-pair's loop as PE filler
  out proj   : out[Sq, 1024] = o_T.T @ WoT  (bf16), DMA to DRAM
  x/w DMA    : inputs and projection weights staged as bf16 (halves HBM
               traffic); K(st>0) / Q(st0, mc>0) projections are spliced
               just-in-time into st0's four head-pair loops so the exp
               stream starts ~26us in instead of ~66us

Scale 1/sqrt(depth) is folded into Wq/bq on the host. bv and bo are folded
into a constant host-side bias (attention rows sum to 1).
"""

import numpy as np

D = 1024
S = 2048
HPC = 8          # heads per core
DH = HPC * 64    # 512 per-core head dims
N_CORES = 8

_CACHE = {}

import os
POP_MOD = int(os.environ.get("K_POP_MOD", "4"))      # pop filler when skc % POP_MOD == POP_MOD-2
VQUARTER = int(os.environ.get("K_VQUARTER", "0"))    # spread V over st0's four blocks
VXBUFS = int(os.environ.get("K_VXBUFS", "3"))
EXBUFS = int(os.environ.get("K_EXBUFS", "8"))
AUXBUFS = int(os.environ.get("K_AUXBUFS", "4"))

# blob layout (bf16 element offsets); assembled by _prepare_in_maps
OFF_XQ = 0
OFF_XK = D * S                      # 2097152
OFF_XV = 2 * D * S
OFF_WQ = 3 * D * S
OFF_WK = OFF_WQ + D * DH
OFF_WV = OFF_WK + D * DH
OFF_WO = OFF_WV + D * DH
OFF_ID = OFF_WO + DH * D
OFF_BQ = OFF_ID + 128 * 128
OFF_BK = OFF_BQ + DH
OFF_MM = OFF_BK + DH                # S*S fp8 bytes as S*S/2 bf16 slots
BLOB_TOT = OFF_MM + S * S // 2


def _build_program(reps=1):
    if reps in _CACHE:
        return _CACHE[reps]

    from concourse import bacc, tile, mybir

    f32 = mybir.dt.float32
    f32r = mybir.dt.float32r
    bf16 = mybir.dt.bfloat16
    f8 = mybir.dt.float8e4
    AF = mybir.ActivationFunctionType

    nc = bacc.Bacc(
        "TRN2",
        target_bir_lowering=False,
        debug=False,
        enable_asserts=False,
        num_devices=N_CORES,
    )

    # All inputs packed into ONE dram blob: per-execution input staging has
    # a large per-buffer fixed cost (~56us each through this PJRT path), so
    # 11 separate inputs cost ~600us before the kernel even starts. The
    # mask's fp8 bytes live in the blob bitcast as bf16 pairs; biases are
    # stored bf16 (|b|~0.01, rounding negligible vs the bf16 matmul noise).
    blob = nc.dram_tensor("blob", [BLOB_TOT], bf16, kind="ExternalInput").ap()
    out = nc.dram_tensor("out", [S, D], bf16, kind="ExternalOutput").ap()

    def seg(off, n):
        return blob[off : off + n]

    xqT = seg(OFF_XQ, D * S)      # flat, row-major [D, S]
    xkT = seg(OFF_XK, D * S)
    xvT = seg(OFF_XV, D * S)
    wqT = seg(OFF_WQ, D * DH)     # flat, row-major [D, DH]
    wkT = seg(OFF_WK, D * DH)
    wvT = seg(OFF_WV, D * DH)
    woT = seg(OFF_WO, DH * D)     # flat, row-major [DH, D]
    ident_d = seg(OFF_ID, 128 * 128).rearrange("(p n) -> p n", p=128)
    bq_d = seg(OFF_BQ, DH)        # bf16
    bk_d = seg(OFF_BK, DH)        # bf16
    mmul = seg(OFF_MM, S * S // 2).bitcast(f8)  # flat f8, row-major [S, S]

    with tile.TileContext(nc) as tc:
        with (
            nc.allow_low_precision(reason="f32r is fp32-width; rounding intended"),
            tc.tile_pool(name="big", bufs=1) as big,
            tc.tile_pool(name="ot", bufs=2) as otp,
            tc.tile_pool(name="wp", bufs=3) as wp,
            tc.tile_pool(name="stream", bufs=2) as stream,
            tc.tile_pool(name="xtp", bufs=2) as xtp,
            tc.tile_pool(name="mp", bufs=2) as mp,
            tc.tile_pool(name="small", bufs=3) as small,
            tc.tile_pool(name="aux", bufs=AUXBUFS) as auxp,
            tc.tile_pool(name="ps", bufs=2, space="PSUM") as psp,
            tc.tile_pool(name="po", bufs=2, space="PSUM") as pop,
        ):
            # ---- persistent tiles ----
            q_dT = big.tile([128, 4, S], bf16, tag="q_dT")
            k_dT = big.tile([128, 4, S], bf16, tag="k_dT")
            v_sb = big.tile([128, 16, HPC, 65], bf16, tag="v_sb")
            woT_sb = big.tile([128, 4, D], bf16, tag="woT_sb")
            bias_q = big.tile([128, 4], f32, tag="bias_q")
            bias_k = big.tile([128, 4], f32, tag="bias_k")
            ident_sb = big.tile([128, 128], bf16, tag="ident_sb")
            warm = big.tile([1, 1], f32, tag="warm")
            nc.vector.memset(warm[:], 0.0)
            # prime the ACT exp table while the DMA front streams in
            nc.scalar.activation(warm[:], warm[:], AF.Exp)
            nc.vector.memset(v_sb[:, :, :, 64:65], 1.0)
            # biases arrive bf16 in the blob; convert to f32 on idle Pool
            bias_qh = big.tile([128, 4], bf16, tag="bias_qh")
            bias_kh = big.tile([128, 4], bf16, tag="bias_kh")
            nc.sync.dma_start(bias_qh[:], bq_d.rearrange("(c p) -> p c", p=128))
            nc.sync.dma_start(bias_kh[:], bk_d.rearrange("(c p) -> p c", p=128))
            nc.gpsimd.tensor_copy(bias_q[:], bias_qh[:])
            nc.gpsimd.tensor_copy(bias_k[:], bias_kh[:])

            for _rep in range(reps):
                # ---- K projection:  k_dT[d, s] = (Wk xk^T)[d, s] + bk[d].
                # Only (st=0, all mc) runs before the attention loop; the
                # other st-chunks are spliced into the first head-pair's
                # skc loop (keys are consumed 128 at a time, so chunk st_k
                # is only needed from skc = 4*st_k onwards). ----
                wk_sb = wp.tile([128, 8, DH], bf16, tag="w", name="wk_sb")
                wr = wkT.rearrange("(kc p m) -> p kc m", p=128, m=DH)
                nc.sync.dma_start(wk_sb[:], wr[:])
                xr = xkT.rearrange("(kc p s) -> p kc s", p=128, s=S)
                xt_tiles = {}

                def emit_kx_dma(st_k):
                    xt = xtp.tile([128, 8, 512], bf16, tag="xt", name=f"xt{st_k}", bufs=3)
                    for h in range(2):
                        nc.sync.dma_start(
                            xt[:, h * 4 : (h + 1) * 4],
                            xr[:, h * 4 : (h + 1) * 4, st_k * 512 : (st_k + 1) * 512],
                        )
                    xt_tiles[st_k] = xt

                emit_kx_dma(0)

                def emit_kproj(st_k, mc):
                    xt = xt_tiles[st_k]
                    ps = pop.tile([128, 512], f32, tag="T", name=f"kp{st_k}_{mc}")
                    for kc in range(8):
                        nc.tensor.matmul(
                            ps[:],
                            wk_sb[:, kc, mc * 128 : (mc + 1) * 128],
                            xt[:, kc],
                            start=(kc == 0),
                            stop=(kc == 7),
                        )
                    # per-partition bias add fused into the PSUM->SBUF
                    # copy on DVE (ACT is saturated by the exp stream)
                    nc.vector.tensor_scalar_add(
                        k_dT[:, mc, st_k * 512 : (st_k + 1) * 512],
                        ps[:],
                        bias_k[:, mc : mc + 1],
                    )

                # ---- Q projection, one Sq-tile at a time. st=0 runs up front;
                # st>0 is emitted inside the previous tile's attention loop so
                # the scheduler can drop its matmuls into PE idle gaps (ACT is
                # the pacer during attention).
                wq_sb = wp.tile([128, 8, DH], bf16, tag="w", name="wq_sb")
                wqr = wqT.rearrange("(kc p m) -> p kc m", p=128, m=DH)
                nc.sync.dma_start(wq_sb[:], wqr[:])
                xq_r = xqT.rearrange("(kc p s) -> p kc s", p=128, s=S)
                qx_tiles = {}
                qp_ps = {}

                def emit_qproj(st, mc, half):
                    # half-size filler unit (~0.85us of PE) so a burst never
                    # exceeds the exp backlog ACT has banked up
                    if mc == 0 and half == 0:
                        qx = stream.tile(
                            [128, 8, 512], bf16, tag="qx", name=f"qx{st}", bufs=2
                        )
                        for kc in range(8):
                            nc.sync.dma_start(
                                qx[:, kc], xq_r[:, kc, st * 512 : (st + 1) * 512]
                            )
                        qx_tiles[st] = qx
                    qx = qx_tiles[st]
                    if half == 0:
                        qp_ps[(st, mc)] = pop.tile(
                            [128, 512], f32, tag="T", name=f"qp{st}_{mc}"
                        )
                    ps = qp_ps[(st, mc)]
                    for kc in range(half * 4, half * 4 + 4):
                        nc.tensor.matmul(
                            ps[:],
                            wq_sb[:, kc, mc * 128 : (mc + 1) * 128],
                            qx[:, kc],
                            start=(kc == 0),
                            stop=(kc == 7),
                        )
                    if half == 1:
                        nc.vector.tensor_scalar_add(
                            q_dT[:, mc, st * 512 : (st + 1) * 512],
                            ps[:],
                            bias_q[:, mc : mc + 1],
                        )

                # ---- V projection: v[sk, dv] (bf16, per-head layout). Emitted
                # as per-sk-chunk groups inside the FIRST attention head-pair's
                # loop (right before the PV matmul that consumes that chunk), so
                # ACT starts the exp stream while V is still being projected.
                wv_sb = wp.tile([128, 8, DH], bf16, tag="w", name="wv_sb")
                wvr = wvT.rearrange("(kc p m) -> p kc m", p=128, m=DH)

                def emit_wv_dma():
                    nc.sync.dma_start(wv_sb[:], wvr[:])

                xv_r = xvT.rearrange("(kc p s) -> p kc s", p=128, s=S)
                vx_tiles = {}

                def emit_vx_dma(sg):
                    vx = stream.tile(
                        [128, 8, 512], bf16, tag="stream", name=f"vx{sg}", bufs=VXBUFS
                    )
                    for kc in range(8):
                        nc.sync.dma_start(
                            vx[:, kc], xv_r[:, kc, sg * 512 : (sg + 1) * 512]
                        )
                    vx_tiles[sg] = vx

                def emit_vproj(skc, hq=None):
                    sg, s4 = divmod(skc, 4)
                    vx = vx_tiles[sg]
                    ps = pop.tile([128, 512], f32, tag="T", name=f"vp{skc}_{hq}")
                    w = 512 if hq is None else 128
                    wsl = wv_sb[:, :, :] if hq is None else wv_sb[:, :, hq * 128 : (hq + 1) * 128]
                    for kc in range(8):
                        nc.tensor.matmul(
                            ps[:, 0:w],
                            vx[:, kc, s4 * 128 : (s4 + 1) * 128],
                            wsl[:, kc],
                            start=(kc == 0),
                            stop=(kc == 7),
                        )
                    dst = v_sb[:, skc, :, :64] if hq is None else v_sb[:, skc, hq * 2 : hq * 2 + 2, :64]
                    nc.vector.tensor_copy(
                        dst,
                        ps[:, 0:w].rearrange("p (h d) -> p h d", h=HPC if hq is None else 2),
                    )

                # ---- output projection for one (st, nh, q4) block; emitted as
                # PE filler inside the NEXT Sq-tile's attention loop ----
                def emit_c_group(st_c, oT_tile, nh, q4, use_act=False):
                    pc = pop.tile([128, 512], f32, tag="T", name=f"pc{st_c}_{nh}_{q4}")
                    for c4 in range(4):
                        nc.tensor.matmul(
                            pc[:],
                            oT_tile[:, c4, q4 * 128 : (q4 + 1) * 128],
                            woT_sb[:, c4, nh * 512 : (nh + 1) * 512],
                            start=(c4 == 0),
                            stop=(c4 == 3),
                        )
                    ob = auxp.tile([128, 512], bf16, tag="ob", name=f"ob{st_c}_{nh}_{q4}")
                    if use_act:
                        # tail-only: ACT is idle once the exp stream has ended
                        nc.scalar.activation(ob[:], pc[:], AF.Identity)
                    else:
                        nc.vector.tensor_copy(ob[:], pc[:])
                    nc.sync.dma_start(
                        out[
                            st_c * 512 + q4 * 128 : st_c * 512 + (q4 + 1) * 128,
                            nh * 512 : (nh + 1) * 512,
                        ],
                        ob[:],
                    )

                mq = mmul.rearrange("(c p s) -> p c s", p=128, s=S)  # [128, 16, S]
                msk_tiles = {}

                def emit_msk_dma(st_m, h2s=(0, 1)):
                    # binary mask is exact in fp8: DMA half the bytes, then
                    # upconvert to bf16 (f8 DVE operands run at half rate)
                    # on the idle Pool. msk bufs=3: the next st's first half
                    # prefetches while the current st's two are still read.
                    for h2 in h2s:
                        mf = mp.tile(
                            [128, 8, 512], f8, tag="mf",
                            name=f"mskf{st_m}_{h2}", bufs=2,
                        )
                        nc.sync.dma_start(
                            mf[:],
                            mq[:, h2 * 8 : (h2 + 1) * 8,
                               st_m * 512 : (st_m + 1) * 512],
                        )
                        mt = mp.tile(
                            [128, 8, 512], bf16, tag="msk",
                            name=f"msk{st_m}_{h2}", bufs=3,
                        )
                        nc.gpsimd.tensor_copy(mt[:], mf[:])
                        msk_tiles.setdefault(st_m, []).append(mt)

                # ---- prologue: only what the first exp needs. K(st=0) and
                # Q(st=0, mc=0) run up front; K(st=1..3) and Q(st=0, mc=1..3)
                # are spliced into st0's skc loops just ahead of the
                # pipelined scores that consume them. st0's mask DMA goes
                # right behind qx0 (mask-mults gate PV in the PE FIFO). ----
                emit_kproj(0, 0)
                emit_qproj(0, 0, 0)
                emit_qproj(0, 0, 1)
                emit_msk_dma(0)
                emit_wv_dma()
                emit_vx_dma(0)
                emit_kx_dma(1)
                emit_vx_dma(1)
                emit_kx_dma(2)
                emit_vx_dma(2)
                emit_kx_dma(3)
                emit_vx_dma(3)
                # out-proj weights + transpose identity aren't needed until
                # well into the first Sq tile; keep them out of the DMA front
                if _rep == 0:
                    nc.sync.dma_start(ident_sb[:], ident_d)
                    woT_r = woT.rearrange("(c p n) -> p c n", p=128, n=D)
                    nc.sync.dma_start(woT_sb[:], woT_r[:])
                # st0 injection schedule, keyed (hp, skc): each K(st_k, mc)
                # unit lands just-in-time for the head-pair that consumes it
                # (hp h's scores need K(st_k, mc=h) by skc = 4*st_k), spreading
                # the projection mountain across all four of st0's loops
                splice = {
                    (0, 0): lambda: emit_kproj(0, 1),
                    (0, 1): lambda: emit_kproj(0, 2),
                    (0, 2): lambda: emit_kproj(0, 3),
                    (0, 3): lambda: emit_kproj(1, 0),
                    (0, 5): lambda: emit_kproj(2, 0),
                    (0, 6): lambda: emit_kproj(1, 1),
                    (0, 8): lambda: emit_kproj(3, 0),
                    (0, 9): lambda: emit_qproj(0, 1, 0),
                    (0, 12): lambda: emit_qproj(0, 1, 1),
                    (1, 0): lambda: emit_kproj(2, 1),
                    (1, 2): lambda: emit_kproj(3, 1),
                    (1, 6): lambda: emit_kproj(1, 2),
                    (1, 9): lambda: emit_qproj(0, 2, 0),
                    (1, 12): lambda: emit_qproj(0, 2, 1),
                    (2, 0): lambda: emit_kproj(2, 2),
                    (2, 2): lambda: emit_kproj(3, 2),
                    (2, 6): lambda: emit_kproj(1, 3),
                    (2, 9): lambda: emit_qproj(0, 3, 0),
                    (2, 12): lambda: emit_qproj(0, 3, 1),
                    (3, 0): lambda: emit_kproj(2, 3),
                    (3, 2): lambda: emit_kproj(3, 3),
                }

                # ---- deferred normalize+transpose for one (st, hp): o_n is
                # already written by the post-loop normalize; here we only
                # run the PE transposes + oT_sb copies (popped as filler so
                # they don't block the next head-pair's scores on PE) ----
                def emit_transpose(st_t, hp_t, o_n, oT_tile):
                    tps = pop.tile(
                        [64, 8, 128], bf16, tag="T", name=f"tp{st_t}_{hp_t}", bufs=2
                    )
                    for par in range(2):
                        for qc in range(4):
                            nc.tensor.transpose(
                                tps[:, par * 4 + qc], o_n[:, par, qc], ident_sb[:]
                            )
                        nc.vector.tensor_copy(
                            oT_tile[par * 64 : (par + 1) * 64, hp_t, :],
                            tps[:, par * 4 : (par + 1) * 4].rearrange(
                                "p a b -> p (a b)"
                            ),
                        )

                # ---- attention per 512-wide Sq tile, with next-tile q-proj,
                # previous-tile output-proj and transposes as PE filler.
                # Scores are software-pipelined ONE skc ahead: they enter the
                # PE FIFO before the current skc's PV matmuls, so the next exp
                # never waits on the exp->mask->PV chain. ----
                prev = None
                filler = []
                sthp = [(st, hp) for st in range(4) for hp in range(4)]
                ps_pend = {}

                def emit_scores(st_s, hp_s, skc_s):
                    ps_s = psp.tile(
                        [128, 1024], f32, tag="ps", name=f"ps{st_s}_{hp_s}_{skc_s}"
                    )
                    sk = slice(skc_s * 128, (skc_s + 1) * 128)
                    sq_s = slice(st_s * 512, (st_s + 1) * 512)
                    for par in range(2):
                        b0 = par * 64
                        nc.tensor.matmul(
                            ps_s[:, par * 512 : (par + 1) * 512],
                            k_dT[b0 : b0 + 64, hp_s, sk],
                            q_dT[b0 : b0 + 64, hp_s, sq_s],
                            start=True,
                            stop=True,
                        )
                    ps_pend[(st_s, hp_s, skc_s)] = ps_s

                oT_tiles = {}
                for idx, (st, hp) in enumerate(sthp):
                    if hp == 0:
                        if st not in msk_tiles:
                            emit_msk_dma(st)
                        elif len(msk_tiles[st]) == 1:
                            # second half (keys 1024-2047): first needed at
                            # skc8, so fetching it here fully hides the DMA
                            emit_msk_dma(st, (1,))
                        oT_tiles[st] = otp.tile(
                            [128, 4, 512], bf16, tag="oT_sb", name=f"oT{st}"
                        )
                    elif hp == 2 and st + 1 < 4 and (st + 1) not in msk_tiles:
                        # prefetch next st's first mask half so the st
                        # boundary's mask-muls don't wait on DMA+upconvert
                        emit_msk_dma(st + 1, (0,))
                    mskh = msk_tiles[st]
                    oT_sb = oT_tiles[st]
                    # o accumulators: [q=128, qc=4, 65] per head; column 64
                    # collects the softmax denominator via v_aug's ones col
                    o_ps = [
                        pop.tile(
                            [128, 4, 65], f32, tag="o",
                            name=f"o{st}_{hp}_{i}", bufs=2,
                        )
                        for i in range(2)
                    ]
                    if idx == 0:
                        emit_scores(st, hp, 0)
                    def emit_pv(s_pv, exs):
                        # PV with P as stationary (128-wide: full PE cols).
                        # o_ps[par][:, qc] += ex_chunk.T @ v_aug
                        for par in range(2):
                            for qc in range(4):
                                # start only on the tile's first group: start
                                # clears has_written for the WHOLE bank, so a
                                # per-group start would wipe the accumulation
                                # state of groups started earlier in the bank
                                nc.tensor.matmul(
                                    o_ps[par][:, qc],
                                    exs[:, par * 512 + qc * 128 : par * 512 + (qc + 1) * 128],
                                    v_sb[:, s_pv, hp * 2 + par],
                                    start=(s_pv == 0 and qc == 0),
                                    stop=(s_pv == 15 and qc == 3),
                                    skip_group_check=True,
                                )

                    ex_map = {}
                    for skc in range(16):
                        if st == 0 and (hp == 0 if not VQUARTER else True):
                            if VQUARTER:
                                emit_vproj(skc, hp)
                            else:
                                emit_vproj(skc)
                        ps_s = ps_pend.pop((st, hp, skc))
                        ex = small.tile([128, 1024], bf16, tag="ex", bufs=EXBUFS)
                        nc.scalar.activation(ex[:], ps_s[:], AF.Exp)
                        # binary mask applied multiplicatively, in place
                        nc.vector.tensor_mul(
                            ex.rearrange("p (t s) -> p t s", t=2),
                            ex.rearrange("p (t s) -> p t s", t=2),
                            mskh[skc // 8][:, skc % 8, None, :].to_broadcast((128, 2, 512)),
                        )
                        ex_map[skc] = ex
                        # spliced K/Q projection units (st0 only), placed at
                        # their LATEST-safe slot so a DMA-gated unit never
                        # parks in the PE FIFO ahead of runnable scores
                        if st == 0 and (hp, skc) in splice:
                            splice[(hp, skc)]()
                        # next scores enter the PE FIFO before this skc's PV
                        if skc < 15:
                            emit_scores(st, hp, skc + 1)
                        elif idx + 1 < len(sthp):
                            nst, nhp = sthp[idx + 1]
                            emit_scores(nst, nhp, 0)
                        if skc % POP_MOD == POP_MOD - 2 and filler:
                            filler.pop(0)()
                        if st == 0 and hp == 0:
                            if skc >= 2:
                                emit_pv(skc - 2, ex_map.pop(skc - 2))
                        else:
                            emit_pv(skc, ex_map.pop(skc))
                    if st == 0 and hp == 0:
                        for s_tail in (14, 15):
                            emit_pv(s_tail, ex_map.pop(s_tail))
                    # normalize: per-q reciprocal of the denominator column,
                    # then per-partition scalar multiply (q is on partitions)
                    o_n = small.tile(
                        [128, 2, 4, 64], bf16, tag="on", name=f"on{st}_{hp}", bufs=2
                    )
                    for par in range(2):
                        rc = auxp.tile([128, 4, 1], f32, tag="rc")
                        nc.vector.reciprocal(rc[:], o_ps[par][:, :, 64:65])
                        nc.vector.tensor_mul(
                            o_n[:, par],
                            o_ps[par][:, :, 0:64],
                            rc.to_broadcast((128, 4, 64)),
                        )
                    filler.append(
                        lambda st=st, hp=hp, o_n=o_n, oT=oT_sb: emit_transpose(
                            st, hp, o_n, oT
                        )
                    )
                    # queue PE filler work; it is popped mid-skc-loop of the
                    # following head pair (ACT-paced steady state has ~40%
                    # PE idle to absorb it)
                    if st < 3:
                        filler.append(lambda st=st, hp=hp: emit_qproj(st + 1, hp, 0))
                        filler.append(lambda st=st, hp=hp: emit_qproj(st + 1, hp, 1))
                    if prev is not None:
                        for j in (2 * hp, 2 * hp + 1):
                            filler.append(
                                lambda p=prev, j=j: emit_c_group(
                                    p[0], p[1], j // 4, j % 4
                                )
                            )
                    if hp == 3:
                        prev = (st, oT_sb)
                while filler:
                    filler.pop(0)()
                for j in range(8):
                    emit_c_group(3, prev[1], j // 4, j % 4, use_act=True)

    nc.compile()
    _CACHE[reps] = nc
    return nc


def _prepare_in_maps(q_in, k_in, v_in, m_in, Wq, bq, Wk, bk, Wv, Wo):
    import ml_dtypes

    bf16 = ml_dtypes.bfloat16
    f8 = ml_dtypes.float8_e4m3
    f32 = np.float32

    ident = np.eye(128, dtype=np.float32).astype(bf16).reshape(-1)
    per_half = []
    for hh in range(2):
        sl = slice(hh * DH, (hh + 1) * DH)
        per_half.append(
            [
                np.ascontiguousarray((Wq[sl, :] / 8.0).T, f32).astype(bf16).reshape(-1),
                np.ascontiguousarray(Wk[sl, :].T, f32).astype(bf16).reshape(-1),
                np.ascontiguousarray(Wv[sl, :].T, f32).astype(bf16).reshape(-1),
                np.ascontiguousarray(Wo[:, sl].T, f32).astype(bf16).reshape(-1),
                ident,
                np.ascontiguousarray(bq[sl] / 8.0, f32).astype(bf16),
                np.ascontiguousarray(bk[sl], f32).astype(bf16),
            ]
        )

    in_maps = []
    for b in range(4):
        xqT = np.ascontiguousarray(q_in[b].T, f32).astype(bf16).reshape(-1)
        xkT = np.ascontiguousarray(k_in[b].T, f32).astype(bf16).reshape(-1)
        xvT = np.ascontiguousarray(v_in[b].T, f32).astype(bf16).reshape(-1)
        maskmul = np.ascontiguousarray((1.0 - m_in[b, 0].T)).astype(f8)
        mask_bf = maskmul.reshape(-1).view(bf16)
        for hh in range(2):
            blob = np.concatenate(
                [xqT, xkT, xvT] + per_half[hh] + [mask_bf]
            )
            assert blob.size == BLOB_TOT
            in_maps.append({"blob": blob})
    return in_maps


def _run(inputs, trace=False, trace_kwargs=None):
    from concourse import bass_utils

    q_in = np.asarray(inputs["q_in"], np.float32)
    k_in = np.asarray(inputs["k_in"], np.float32)
    v_in = np.asarray(inputs["v_in"], np.float32)
    m_in = np.asarray(inputs["m_in"], np.float32)
    Wq = np.asarray(inputs["Wq"], np.float32)
    bq = np.asarray(inputs["bq"], np.float32)
    Wk = np.asarray(inputs["Wk"], np.float32)
    bk = np.asarray(inputs["bk"], np.float32)
    Wv = np.asarray(inputs["Wv"], np.float32)
    bv = np.asarray(inputs["bv"], np.float32)
    Wo = np.asarray(inputs["Wo"], np.float32)
    bo = np.asarray(inputs["bo"], np.float32)

    nc = _build_program()
    in_maps = _prepare_in_maps(q_in, k_in, v_in, m_in, Wq, bq, Wk, bk, Wv, Wo)
    kw = {}
    if trace:
        kw["trace"] = True
        if trace_kwargs:
            kw["trace_kwargs"] = trace_kwargs
    res = bass_utils.run_bass_kernel_spmd(
        nc, in_maps, core_ids=list(range(N_CORES)), **kw
    )

    total_bias = (bo + bv @ Wo.T).astype(np.float32)
    output = np.empty((4, S, D), np.float32)
    for b in range(4):
        output[b] = res.results[2 * b]["out"].astype(np.float32)
        output[b] += res.results[2 * b + 1]["out"].astype(np.float32)
        output[b] += total_bias
    return output, res


def kernel(**inputs) -> np.ndarray:
    output, _ = _run(inputs, trace=False)
    return output


def run_traced(inputs):
    """For test.py: returns (output, BassKernelResults with exec_time_ns)."""
    return _run(inputs, trace=True)

